# revision 29
# baseline (speedup 1.0000x reference)
"""Multi-head attention (B=4, T=2048, D=1024, H=16, causal) on 8 trn2 NeuronCores.

Sharding: core c handles batch b = c//2 and an interleaved half of the
queries, as 4 chunks of 256 rows (half 0 -> q-chunks {0, 3, 4, 7},
half 1 -> {1, 2, 5, 6}; the assignment balances causal work: both halves
attend 36 k-tiles total).  Each core computes the FULL K/V projections
for its batch locally (cheaper than exchanging K/V), so there is no
inter-core communication at all and the host just concatenates per-core
output column blocks.

All 8 cores run ONE program (one dispatch): the attention loop processes
four chunk slots with fixed k-tile counts (4, 8, 12, 16) — per slot the
superset of both halves' causal needs — and every per-core difference
(which tiles are causal-masked / fully masked / padding; always the last
4 k-tiles of a slot) lives in a per-core bf16 multiplicative mask tensor
applied to P on the vector engine.

Numerics: activations/weights ship as bf16; matmuls run bf16 (full PE
rate, 512-wide moving operands — matmul PSUM writes cannot span banks)
accumulating in fp32 PSUM; softmax statistics stay fp32/f32r.  On-chip layout keeps activations transposed
([d, tokens]) so every matmul operand is K-major:
    Q^T = Wq^T.T @ X^T           (per 128-d' tile, accumulated in PSUM)
    S^T[k,q] = (K^T slice).T @ Q^T slice      (contraction d_k = 64)
    P^T = exp(S^T / 8) (bf16) * mask
    [x^T | s] = V_aug.T @ P^T    (V augmented with a ones column -> row sums)
    x^T normalized by s via DVE reciprocal + PE outer-product replicate
    out^T = Wo^T.T @ x^T + b_o, DMA'd straight to the core's column block.
"""
import numpy as np
from contextlib import ExitStack

import ml_dtypes

import concourse.bass as bass
import concourse.tile as tile
import concourse.mybir as mybir
from concourse.bass_utils import run_bass_kernel_spmd
from bass_rust import ScopedClock

f32 = mybir.dt.float32
f32r = mybir.dt.float32r
bf16 = mybir.dt.bfloat16
u16 = mybir.dt.uint16
EXPF = mybir.ActivationFunctionType.Exp

B, T, D = 4, 2048, 1024
H, DK = 16, 64
N_CORES = 8
TQ = 1024                      # queries per core
CH = 256                       # query-chunk width
NSLOT = 4                      # chunk slots per core; slot ci spans 4*(ci+1) k-tiles
HALF_CHUNKS = ((0, 3, 4, 7), (1, 2, 5, 6))  # global q-chunk per (half, slot)

_MODE_MAP = {"sem-ge-imm": "sem-ge", "sem-eq-imm": "sem-eq", "sem-le-imm": "sem-le"}


def _patched_drain_and_barrier(self, tick_clock, wait_clock):
    # This walrus build rejects Drain/NoOp instructions that carry sync
    # waits ("Too many sync wait commands"), which the stock Tile tail
    # emits. Put the tail waits on individual EventSemaphore instructions
    # and use sem-only barriers instead of the drain butterfly.
    nc = self.nc
    collector = nc.sync.nop(nofuse=True, hint="tile_tail_wait")
    wait_clock.add_sem_waits(collector.ins, ScopedClock({None: tick_clock.global_clock}))
    si = collector.ins.sync_info
    waits = list(si.on_wait) if si else []
    if si:
        collector.ins.sync_info = mybir.SyncInfo(on_wait=[], on_update=[])
    assert self.sems is not None
    name2sem = {s.name: s for s in self.sems.allocated().values()}
    for w in waits:
        nc.sync.wait_op(name2sem[w.ant_name], w.wait_value, _MODE_MAP.get(w.wait_mode, "sem-ge"))
    nc.all_engine_barrier(sem_only=True)
    popped = nc._tile_sem_poison_stack.pop()
    assert popped is self._sem_poison
    nc.clear_and_free_semaphores(list(self.sems.allocated().values()))
    nc.all_engine_barrier(sem_only=True)


tile.TileContext._drain_and_barrier = _patched_drain_and_barrier


def _fixup_sync_waits(nc):
    """This walrus build accepts at most 1 sync wait per compute/DMA
    instruction (EventSemaphore: 2). Tile's add_semaphores can emit more.
    Hoist excess waits onto EventSemaphore instructions inserted just
    before the over-budget instruction on the same engine."""
    for bb in nc.main_func.blocks:
        insts = bb.instructions
        out = []
        changed = False
        for ins in insts:
            si = ins.sync_info
            cap = 2 if type(ins).__name__ == "InstEventSemaphore" else 1
            if si is not None and len(si.on_wait) > cap:
                waits = list(si.on_wait)
                keep, excess = waits[-1:], waits[:-1]
                for i in range(0, len(excess), 2):
                    ev = mybir.InstEventSemaphore(
                        name=nc.get_next_instruction_name(),
                        ins=[], outs=[],
                        sync_info=mybir.SyncInfo(on_wait=excess[i:i + 2], on_update=[]),
                    )
                    ev.engine = ins.engine
                    out.append(ev)
                ins.sync_info = mybir.SyncInfo(on_wait=keep, on_update=list(si.on_update))
                changed = True
            out.append(ins)
        if changed:
            bb.instructions = out


def _emit_kernel(nc):
    qT = nc.dram_tensor("qT", [D, TQ], bf16, kind="ExternalInput")
    kT = nc.dram_tensor("kT", [D, T], bf16, kind="ExternalInput")
    vT = nc.dram_tensor("vT", [D, T], bf16, kind="ExternalInput")
    wq = nc.dram_tensor("wqT", [D, D], bf16, kind="ExternalInput")
    wk = nc.dram_tensor("wkT", [D, D], bf16, kind="ExternalInput")
    wv = nc.dram_tensor("wvT", [D, D], bf16, kind="ExternalInput")
    wo = nc.dram_tensor("woT", [D, D], bf16, kind="ExternalInput")
    bq = nc.dram_tensor("bq", [128, 8], f32, kind="ExternalInput")
    bk = nc.dram_tensor("bk", [128, 8], f32, kind="ExternalInput")
    bv = nc.dram_tensor("bv", [128, D], f32, kind="ExternalInput")
    bo = nc.dram_tensor("bo", [128, 8], f32, kind="ExternalInput")
    # per-core causal/padding masks for the P^T tiles that differ between
    # the two halves: the LAST quad of k-tiles of each slot (slot ci ->
    # k-tiles 4*ci .. 4*ci+3), each [128 keys, 256 queries]; storage row
    # r holds k-tile r's mask
    msk = nc.dram_tensor("msk", [128, 16 * CH], bf16, kind="ExternalInput")
    outT = nc.dram_tensor("outT", [D, TQ], f32, kind="ExternalOutput")

    with tile.TileContext(nc, num_cores=N_CORES) as tc, ExitStack() as ctx:
        const = ctx.enter_context(tc.tile_pool(name="const", bufs=1))
        perm = ctx.enter_context(tc.tile_pool(name="perm", bufs=1))
        xtp = ctx.enter_context(tc.tile_pool(name="xtp", bufs=2))

        # Persistent on-chip tensors: [p, i, t] = full[i*128+p, t]
        QT = perm.tile([128, 8, TQ], bf16)
        KT = perm.tile([128, 8, T], bf16)
        Vg = perm.tile([128, 16, H * 65], bf16)   # per k-tile: 16 heads x (64 + 1 one)
        wot = perm.tile([128, 8, D], bf16)        # Wo^T, persists through attention

        bq_t = const.tile([128, 8], f32)
        bk_t = const.tile([128, 8], f32)
        bv_t = const.tile([128, D], f32)
        bo_t = const.tile([128, 8], f32)
        msk_t = const.tile([128, 16 * CH], bf16)
        ones_t = const.tile([65, 64], f32r)
        nc.vector.memset(ones_t[:].bitcast(f32), 1.0)
        nc.sync.dma_start(bq_t[:], bq[:])
        nc.sync.dma_start(bk_t[:], bk[:])
        nc.sync.dma_start(bv_t[:], bv[:])
        nc.sync.dma_start(bo_t[:], bo[:])
        nc.sync.dma_start(msk_t[:], msk[:])
        # ones column of V_aug (columns 64 + 65*n, uniform stride)
        nc.vector.memset(
            Vg[:].rearrange("p i (h j) -> p (i h) j", j=65)[:, :, 64:65]
            .bitcast(u16), 0x3F80)

        # PE warmup: dependency-free matmuls that fill the initial input-DMA
        # wait so the HAM clock gate is released before the real work starts.
        with tc.tile_pool(name="warm", bufs=1) as warm, \
                tc.tile_pool(name="warm_psum", bufs=2, space="PSUM") as warm_psum:
            wrm = warm.tile([64, 512], f32r)
            nc.vector.memset(wrm[:].bitcast(f32), 0.0)
            for _ in range(28):
                wp = warm_psum.tile([64, 512], f32)
                nc.tensor.matmul(wp[:], ones_t[0:64, :], wrm[:])

        # ---------------- projections ----------------
        with ExitStack() as ph:
            wpool = ph.enter_context(tc.tile_pool(name="wproj", bufs=2))
            xpool = ph.enter_context(tc.tile_pool(name="xchunk", bufs=2))
            p_psum = ph.enter_context(tc.tile_pool(name="p_psum", bufs=4, space="PSUM"))

            # Q / K: outputs stay transposed ([d', tokens])
            for name, wdram, xdram, dst, bias, ntok in (
                ("q", wq, qT, QT, bq_t, TQ),
                ("k", wk, kT, KT, bk_t, T),
            ):
                wt = wpool.tile([128, 8, D], bf16, tag="wproj")
                wsrc = wdram.rearrange("(i p) n -> p i n", p=128)
                for kt in range(8):
                    nc.sync.dma_start(wt[:, kt, :], wsrc[:, kt, :])
                xsrc = xdram.rearrange("(i p) t -> p i t", p=128)
                for tck in range(ntok // 1024):
                    xc = xpool.tile([128, 8, 1024], bf16, tag="xchunk")
                    for kt in range(8):
                        nc.sync.dma_start(xc[:, kt, :], xsrc[:, kt, tck * 1024:(tck + 1) * 1024])
                    for e in range(8):
                        ps = p_psum.tile([128, 1024], f32, tag="pp")
                        for sub in range(2):
                            for kt in range(8):
                                nc.tensor.matmul(
                                    ps[:, sub * 512:(sub + 1) * 512],
                                    wt[:, kt, e * 128:(e + 1) * 128],
                                    xc[:, kt, sub * 512:(sub + 1) * 512],
                                    start=(kt == 0), stop=(kt == 7),
                                )
                        nc.vector.tensor_add(
                            dst[:, e, tck * 1024:(tck + 1) * 1024], ps[:],
                            bias[:, e:e + 1].to_broadcast((128, 1024)),
                        )

            # V: natural layout (tokens in partitions), into V_aug
            wvt = wpool.tile([128, 8, D], bf16, tag="wproj")
            nc.sync.dma_start(wvt[:], wv.rearrange("(i p) n -> p i n", p=128))
            vsrc = vT.rearrange("(i p) t -> p i t", p=128)
            bv3 = bv_t[:].rearrange("p (h j) -> p h j", h=H)
            for tg in range(2):
                xc = xpool.tile([128, 8, 1024], bf16, tag="xchunk")
                for kt in range(8):
                    nc.sync.dma_start(xc[:, kt, :], vsrc[:, kt, tg * 1024:(tg + 1) * 1024])
                for tt in range(8):
                    ps = p_psum.tile([128, 1024], f32, tag="pp")
                    for kt in range(8):
                        for sub in range(2):
                            nc.tensor.matmul(
                                ps[:, sub * 512:(sub + 1) * 512],
                                xc[:, kt, tt * 128:(tt + 1) * 128],
                                wvt[:, kt, sub * 512:(sub + 1) * 512],
                                start=(kt == 0), stop=(kt == 7),
                            )
                    ti = tg * 8 + tt
                    nc.vector.tensor_add(
                        Vg[:, ti, :].rearrange("p (h j) -> p h j", h=H)[:, :, 0:64],
                        ps[:].rearrange("p (h j) -> p h j", h=H),
                        bv3,
                    )

            # Wo^T load overlaps the projections
            wosrc = wo.rearrange("(i p) n -> p i n", p=128)
            for kt in range(8):
                nc.sync.dma_start(wot[:, kt, :], wosrc[:, kt, :])

        # ---------------- attention + interleaved output projection ----------------
        with ExitStack() as ph:
            opool = ph.enter_context(tc.tile_pool(name="opool", bufs=3))
            ppool = ph.enter_context(tc.tile_pool(name="ppool", bufs=20))
            rpool = ph.enter_context(tc.tile_pool(name="rpool", bufs=3))
            s_psum = ph.enter_context(tc.tile_pool(name="s_psum", bufs=2, space="PSUM"))
            pv_psum = ph.enter_context(tc.tile_pool(name="pv_psum", bufs=2, space="PSUM"))

            def emit_outproj(xt, ci):
                # out columns [ci*CH, (ci+1)*CH) of this core's outT
                for e in range(8):
                    ps = s_psum.tile([128, CH], f32, tag="sdiag")
                    for kt in range(8):
                        nc.tensor.matmul(
                            ps[:],
                            wot[:, kt, e * 128:(e + 1) * 128],
                            xt[:, kt, :],
                            start=(kt == 0), stop=(kt == 7),
                        )
                    ot = opool.tile([128, CH], f32, tag="otile")
                    nc.vector.tensor_add(
                        ot[:], ps[:], bo_t[:, e:e + 1].to_broadcast((128, CH))
                    )
                    nc.sync.dma_start(
                        outT[e * 128:(e + 1) * 128, ci * CH:(ci + 1) * CH], ot[:]
                    )

            pv_q = []    # (ptiles, h, xt): S/exp/mask emitted, PV pending
            nm_q = []    # (pv, rr, po, hi, xt): PV emitted, normalize pending

            def emit_normalize(pv, rr, po, hi, xt):
                rp = s_psum.tile([64, CH], f32, tag="sdiag")
                nc.tensor.matmul(rp[:], ones_t[64:65, :], rr[64:65, :])
                nc.vector.tensor_copy(xt[po:po + 64, hi, :], pv[0:64, :])
                nc.vector.tensor_mul(
                    xt[po:po + 64, hi, :],
                    xt[po:po + 64, hi, :],
                    rp[:],
                )

            def emit_pv(ptiles, h, xt):
                po = 64 * (h % 2)
                hi = h // 2
                nkt = len(ptiles)
                pv = pv_psum.tile([65, CH], f32)
                for kt in range(nkt):
                    nc.tensor.matmul(
                        pv[:],
                        Vg[:, kt, 65 * h:65 * (h + 1)],
                        ptiles[kt][:],
                        start=(kt == 0), stop=(kt == nkt - 1),
                    )
                rr = rpool.tile([65, CH], f32r, tag="rrow")
                with nc.allow_low_precision(reason="softmax denom recip in f32r"):
                    nc.vector.reciprocal(rr[64:65, :], pv[64:65, :])
                return (pv, rr, po, hi, xt)

            # Three-deep software pipeline over the 64 (slot, head) blocks:
            # the PE stream is [S(0)] [S(1)] [S(2) PV(0)] [S(3) PV(1)
            # norm(0)] ... so the PE never waits on the exps (ACT) or the
            # reciprocal (DVE) of the block it just produced.
            msk2 = msk_t[:]  # [128, 16*CH]; CH-wide row r = k-tile r's mask
            for ci in range(NSLOT):
                nkt = 4 * (ci + 1)     # k-tiles this slot attends to
                q0 = ci * CH           # column offset into QT for this slot
                xt = xtp.tile([128, 8, CH], bf16, tag="xt")
                for h in range(H):
                    po = 64 * (h % 2)
                    hi = h // 2
                    ptiles = []
                    for m in range(nkt // 4):
                        # quad of 4 S tiles -> one 2-bank [128, 4*CH] PSUM
                        # span -> one exp; the LAST quad (k-tiles 4*ci ..
                        # 4*ci+3, the only tiles whose mask differs between
                        # the halves) gets the mask multiply
                        sp4 = s_psum.tile([128, 4 * CH], f32, tag="spair")
                        pt4 = ppool.tile([128, 4 * CH], bf16, tag="ppair", bufs=12)
                        for sub in range(4):
                            kt = 4 * m + sub
                            nc.tensor.matmul(
                                sp4[:, sub * CH:(sub + 1) * CH],
                                KT[po:po + 64, hi, kt * 128:(kt + 1) * 128],
                                QT[po:po + 64, hi, q0:q0 + CH],
                            )
                        nc.scalar.activation(pt4[:], sp4[:], EXPF, scale=0.125)
                        if m == nkt // 4 - 1:
                            nc.vector.tensor_mul(
                                pt4[:],
                                pt4[:],
                                msk2[:, 4 * ci * CH:(4 * ci + 4) * CH],
                            )
                        for sub in range(4):
                            ptiles.append(pt4[:, sub * CH:(sub + 1) * CH])
                    pv_q.append((ptiles, h, xt))
                    if len(pv_q) > 2:
                        nm_q.append(emit_pv(*pv_q.pop(0)))
                        if len(nm_q) > 1:
                            emit_normalize(*nm_q.pop(0))

                # group flush: finish every block of this slot so xt is
                # final, then emit the output projection for it
                while pv_q:
                    nm_q.append(emit_pv(*pv_q.pop(0)))
                while nm_q:
                    emit_normalize(*nm_q.pop(0))
                emit_outproj(xt, ci)


_NC_CACHE = None


def _build_nc():
    global _NC_CACHE
    if _NC_CACHE is None:
        nc = bass.Bass("TRN2", target_bir_lowering=False, debug=False, num_devices=N_CORES)
        _emit_kernel(nc)
        _fixup_sync_waits(nc)
        _NC_CACHE = nc
    return _NC_CACHE


def _check_masks(attention_mask, key_padding_mask):
    # The kernel exploits the causal structure; verify the runtime masks
    # actually match it (they do for this problem's setup_inputs()).
    am = np.asarray(attention_mask)[0]
    causal = np.triu(np.ones((T, T), np.int32), k=1)
    if not np.array_equal(am != 0, causal != 0):
        raise ValueError("kernel specialised for strict-upper-triangular causal mask")
    if np.asarray(key_padding_mask).any():
        raise ValueError("kernel specialised for all-attendable key_padding_mask")


def _host_masks(half):
    """[128, 16*CH] bf16: CH-wide row r holds k-tile r's mask, applied by
    slot r//4 (whose last quad is k-tiles 4*(r//4) .. 4*(r//4)+3):
    mask[k_local, q_local] = 1 iff q_global >= k_global."""
    out = np.zeros((128, 16, CH), np.float32)
    for row in range(16):
        g = HALF_CHUNKS[half][row // 4]
        qg = g * CH + np.arange(CH)[None, :]
        kg = row * 128 + np.arange(128)[:, None]
        out[:, row, :] = (qg >= kg).astype(np.float32)
    return np.ascontiguousarray(out.reshape(128, 16 * CH)).astype(ml_dtypes.bfloat16)


def _qcols(half):
    return np.concatenate([np.arange(g * CH, (g + 1) * CH) for g in HALF_CHUNKS[half]])


def _make_in_maps(inputs):
    to_bf = lambda a: np.ascontiguousarray(np.asarray(a, np.float32).astype(ml_dtypes.bfloat16))
    query = np.asarray(inputs["query"], np.float32)
    key = np.asarray(inputs["key"], np.float32)
    value = np.asarray(inputs["value"], np.float32)
    W = {n: np.asarray(inputs[n], np.float32) for n in ("W_q", "W_k", "W_v", "W_o")}
    b = {n: np.asarray(inputs[n], np.float32) for n in ("b_q", "b_k", "b_v", "b_o")}
    _check_masks(inputs["attention_mask"], inputs["key_padding_mask"])

    wq_t = to_bf(W["W_q"].T)
    wk_t = to_bf(W["W_k"].T)
    wv_t = to_bf(W["W_v"].T)
    wo_t = to_bf(W["W_o"].T)
    bq_t = np.ascontiguousarray(b["b_q"].reshape(8, 128).T)
    bk_t = np.ascontiguousarray(b["b_k"].reshape(8, 128).T)
    bv_t = np.tile(b["b_v"][None, :], (128, 1))
    bo_t = np.ascontiguousarray(b["b_o"].reshape(8, 128).T)
    masks = [_host_masks(0), _host_masks(1)]
    kT_c = [to_bf(key[bb].T) for bb in range(B)]
    vT_c = [to_bf(value[bb].T) for bb in range(B)]

    in_maps = []
    for c in range(N_CORES):
        bb, half = c // 2, c % 2
        qcols = _qcols(half)
        in_maps.append({
            "qT": to_bf(query[bb].T[:, qcols]),
            "kT": kT_c[bb],
            "vT": vT_c[bb],
            "wqT": wq_t, "wkT": wk_t, "wvT": wv_t, "woT": wo_t,
            "bq": bq_t, "bk": bk_t, "bv": bv_t, "bo": bo_t,
            "msk": masks[half],
        })
    return in_maps


def kernel(**inputs):
    nc = _build_nc()
    in_maps = _make_in_maps(inputs)
    res = run_bass_kernel_spmd(nc, in_maps, core_ids=list(range(N_CORES)))
    out = np.empty((B, T, D), np.float32)
    for bb in range(B):
        out[bb, _qcols(0)] = res.results[2 * bb]["outT"].T
        out[bb, _qcols(1)] = res.results[2 * bb + 1]["outT"].T
    return out
